# revision 1
# baseline (speedup 1.0000x reference)
"""ChebNet (K=3, L=2) forward on 8 Trainium2 NeuronCores.

Node-sharded SPMD: each core owns 6250 dst rows. Chebyshev recurrence in the
U-basis (U1 = A_hat U0, U2 = A_hat U1; out = U0(W0-W2) - U1 W1 + 2 U2 W2).
SpMM = dma_gather of per-edge feature rows from a replicated DRAM table (two
int16-addressable halves) + PE mask-matmuls with on-device-generated masks
carrying norm[src]*norm[dst] edge weights. Fixed windows of 32 dst rows,
6 x 128 edge slots per window (3 per table half), 8 windows per PSUM chunk.
Full node tables are rebuilt between SpMMs with AllGather (3 per pass).
"""
import os
import numpy as np

N, E, D, K, L = 50000, 800000, 64, 3, 2
NCORES = 8
NSH = N // NCORES              # 6250 dst rows per core
WIN = 32                       # dst rows per window
NWIN = 208                     # windows per core -> 6656 padded cols
NCOLS = NWIN * WIN
CWIN = 8                       # windows per PSUM chunk
NCH = NWIN // CWIN             # 26 chunks
CCOLS = CWIN * WIN             # 256 cols per chunk
TPW = 3                        # tiles per window per table half
ROWS_PAD = NCOLS               # padded rows per core block in the table
TBL_ROWS = NCORES * ROWS_PAD   # 53248
HALF = TBL_ROWS // 2           # 26624 (< 32768: int16-reachable)
NTILE_CH = CWIN * 2 * TPW      # 48 mask tiles per chunk
SLOTS_CH = CWIN * 2 * TPW * 128  # 6144 slots per chunk
SLOTS_TOT = NCH * SLOTS_CH     # 159744
NTILE_TOT = SLOTS_TOT // 128   # 1248
GIDX = 1024                    # rows per dma_gather call
GPC = SLOTS_CH // 2 // GIDX    # 3 gathers per half per chunk

_CACHE = {}


# ---------------------------------------------------------------------------
# Workaround for this walrus build: any instruction carrying >1 sync wait is
# rejected ("Too many sync wait commands"). Hoist extras onto 1-wait NoOps on
# the same engine (per-engine program order preserves semantics).
_ws_counter = [0]


def _split_multiwaits(nc):
    import concourse.mybir as mybir
    n_split = 0
    for fn in nc.m.functions:
        for bb in fn.blocks:
            new_list = []
            changed = False
            for inst in bb.instructions:
                si = inst.sync_info
                waits = list(si.on_wait) if si is not None else []
                if len(waits) > 1:
                    changed = True
                    for w in waits[:-1]:
                        _ws_counter[0] += 1
                        nop = mybir.InstNoOp(
                            name=f"waitsplit-{_ws_counter[0]}",
                            ins=[], outs=[],
                            sync_info=mybir.SyncInfo(on_wait=[w], on_update=[]),
                        )
                        nop.engine = inst.engine
                        nc.register_instruction(nop, overwrite=True)
                        new_list.append(nop)
                        n_split += 1
                    si.on_wait = waits[-1:]
                new_list.append(inst)
            if changed:
                bb.instructions[:] = new_list
    return n_split


def _finalize_with_split(nc):
    import concourse.bass as _bass
    nc.compile()           # Bacc passes (incl. library-load insertion)
    _split_multiwaits(nc)  # after replace_nops_with_events, before freeze
    _bass.Bass.finalize(nc)


def _build_runner(nc, n_cores):
    """SPMD runner over the axon PJRT backend (keeps the jitted executable
    and device-resident inputs so repeat calls can be timed)."""
    import jax
    from jax.sharding import Mesh, PartitionSpec
    from jax.experimental.shard_map import shard_map
    import concourse.mybir as mybir
    from concourse.bass2jax import (
        _bass_exec_p, install_neuronx_cc_hook, partition_id_tensor)

    install_neuronx_cc_hook()
    partition_name = nc.partition_id_tensor.name if nc.partition_id_tensor else None

    in_names, out_names, out_avals, zero_outs = [], [], [], []
    for alloc in nc.m.functions[0].allocations:
        if not isinstance(alloc, mybir.MemoryLocationSet):
            continue
        name = alloc.memorylocations[0].name
        if alloc.kind == "ExternalInput":
            if name != partition_name:
                in_names.append(name)
        elif alloc.kind == "ExternalOutput":
            shape = tuple(alloc.tensor_shape)
            dtype = mybir.dt.np(alloc.dtype)
            out_names.append(name)
            out_avals.append(jax.core.ShapedArray(shape, dtype))
            zero_outs.append(np.zeros(shape, dtype))
    n_params = len(in_names)
    all_in_names = list(in_names) + list(out_names)
    if partition_name is not None:
        all_in_names.append(partition_name)

    def _body(*args):
        operands = list(args)
        if partition_name is not None:
            operands.append(partition_id_tensor())
        outs = _bass_exec_p.bind(
            *operands,
            out_avals=tuple(out_avals),
            in_names=tuple(all_in_names),
            out_names=tuple(out_names),
            lowering_input_output_aliases=(),
            sim_require_finite=True,
            sim_require_nnan=True,
            nc=nc,
        )
        return tuple(outs)

    devices = jax.devices()[:n_cores]
    mesh = Mesh(np.asarray(devices), ("core",))
    in_specs = (PartitionSpec("core"),) * (n_params + len(out_names))
    out_specs = (PartitionSpec("core"),) * len(out_names)
    sharded = jax.jit(
        shard_map(_body, mesh=mesh, in_specs=in_specs, out_specs=out_specs,
                  check_rep=False),
        keep_unused=True,
    )

    def run(in_maps, iters=1):
        import time as _time
        per_core = [[np.asarray(m[name]) for name in in_names] for m in in_maps]
        concat_in = [
            np.concatenate([per_core[c][i] for c in range(n_cores)], axis=0)
            for i in range(n_params)
        ]
        concat_zeros = [
            np.zeros((n_cores * z.shape[0], *z.shape[1:]), z.dtype)
            for z in zero_outs
        ]
        sharding = jax.sharding.NamedSharding(mesh, PartitionSpec("core"))
        dev_in = [jax.device_put(a, sharding) for a in concat_in + concat_zeros]
        out = sharded(*dev_in)
        jax.block_until_ready(out)
        times = []
        for _ in range(iters):
            t0 = _time.perf_counter()
            out = sharded(*dev_in)
            jax.block_until_ready(out)
            times.append(_time.perf_counter() - t0)
        results = [
            {name: np.asarray(out[i]).reshape(n_cores, *out_avals[i].shape)[c]
             for i, name in enumerate(out_names)}
            for c in range(n_cores)
        ]
        return results, times

    return run


def _host_prep(features, src, dst, W, b, pw, pb):
    src = np.asarray(src).astype(np.int64)
    dst = np.asarray(dst).astype(np.int64)
    features = np.asarray(features, dtype=np.float32)
    W = np.asarray(W, dtype=np.float32)
    b = np.asarray(b, dtype=np.float32)
    pw = np.asarray(pw, dtype=np.float32).reshape(D, 1)
    pb = np.asarray(pb, dtype=np.float32).reshape(1)

    deg = np.bincount(dst, minlength=N).astype(np.float32)
    norm = np.clip(deg, 1.0, None) ** -0.5
    wedge = (norm[src] * norm[dst]).astype(np.float32)

    Wflat = np.zeros((D, L * 3 * D), dtype=np.float32)
    for l in range(L):
        for t, Wt in enumerate((W[l, 0] - W[l, 2], -W[l, 1], 2.0 * W[l, 2])):
            Wflat[:, (l * 3 + t) * D:(l * 3 + t + 1) * D] = Wt

    feat_pad = np.zeros((TBL_ROWS, D), dtype=np.float32)
    for j in range(NCORES):
        feat_pad[ROWS_PAD * j:ROWS_PAD * j + NSH] = features[NSH * j:NSH * (j + 1)]

    src_pad = (src // NSH) * ROWS_PAD + (src % NSH)
    core_of = dst // NSH
    dl_all = dst - core_of * NSH

    in_maps = []
    for i in range(NCORES):
        sel = core_of == i
        e_src = src_pad[sel]
        e_dl = dl_all[sel]
        e_w = wedge[sel]
        win = e_dl // WIN
        col = (e_dl % WIN).astype(np.float32)
        half = (e_src >= HALF).astype(np.int64)

        # order edges by (window, half); compute slot within (window, half)
        key = win * 2 + half
        order = np.argsort(key, kind="stable")
        ks = key[order]
        # rank within group
        grp_start = np.searchsorted(ks, np.arange(NWIN * 2), side="left")
        grp_cnt = np.diff(np.append(grp_start, ks.size))
        if grp_cnt.max() > TPW * 128:
            raise RuntimeError(f"window overflow core {i}: {grp_cnt.max()}")
        rank = np.arange(ks.size) - grp_start[ks]

        ww = win[order]
        hh = half[order]
        c_ = ww // CWIN
        wl = ww % CWIN
        slot = (c_ * SLOTS_CH + hh * (SLOTS_CH // 2) + wl * (TPW * 128) + rank)

        idx_slots = np.zeros(SLOTS_TOT, dtype=np.int16)
        col_slots = np.zeros(SLOTS_TOT, dtype=np.float32)
        w_slots = np.zeros(SLOTS_TOT, dtype=np.float32)
        idx_slots[slot] = (e_src[order] - hh * HALF).astype(np.int16)
        col_slots_tmp = col[order]
        w_slots_tmp = e_w[order]

        # mask tile layout: tile index mt within chunk = wl*(2*TPW) + hh*TPW + rank//128
        tj = rank // 128
        part = rank % 128
        mt = c_ * NTILE_CH + wl * (2 * TPW) + hh * TPW + tj
        col_slots[mt * 128 + part] = col_slots_tmp
        w_slots[mt * 128 + part] = w_slots_tmp

        # gather index wrap: gather g slot s -> idx_arr[s%16, g*64 + s//16]
        wrapped = idx_slots.reshape(SLOTS_TOT // GIDX, 64, 16).transpose(0, 2, 1)
        idx_arr16 = wrapped.reshape(SLOTS_TOT // GIDX, 16, 64)
        idx_arr = np.zeros((16, SLOTS_TOT // 16), dtype=np.int16)
        for g in range(SLOTS_TOT // GIDX):
            idx_arr[:, g * 64:(g + 1) * 64] = idx_arr16[g]
        idx_arr = np.tile(idx_arr, (8, 1))

        dcol = col_slots.reshape(NTILE_TOT, 128).T.copy()
        wval = w_slots.reshape(NTILE_TOT, 128).T.copy()

        f0T = np.zeros((D, NCOLS), dtype=np.float32)
        f0T[:, :NSH] = features[NSH * i:NSH * (i + 1)].T

        iota = np.tile(np.arange(WIN, dtype=np.float32)[None, :], (128, 1))

        in_maps.append({
            "feat_pad": feat_pad,
            "f0T": f0T,
            "idx_all": idx_arr,
            "dcol": dcol,
            "wval": wval,
            "iota": iota,
            "Wflat": Wflat,
            "bvec": b.T.copy(),
            "pwv": pw,
            "pbv": pb.reshape(1, 1),
        })
    return in_maps


def _build_nc(repeat=1, mode="full"):
    import concourse.bacc as bacc
    import concourse.mybir as mybir
    import concourse.tile as tile
    from concourse.masks import make_identity
    f32 = mybir.dt.float32

    nc = bacc.Bacc("TRN2", num_swdge_queues=4)
    feat_pad = nc.declare_dram_parameter("feat_pad", [TBL_ROWS, D], f32, isOutput=False)
    f0T_in = nc.declare_dram_parameter("f0T", [D, NCOLS], f32, isOutput=False)
    idx_in = nc.declare_dram_parameter("idx_all", [128, SLOTS_TOT // 16], mybir.dt.int16, isOutput=False)
    dcol_in = nc.declare_dram_parameter("dcol", [128, NTILE_TOT], f32, isOutput=False)
    wval_in = nc.declare_dram_parameter("wval", [128, NTILE_TOT], f32, isOutput=False)
    iota_in = nc.declare_dram_parameter("iota", [128, WIN], f32, isOutput=False)
    W_in = nc.declare_dram_parameter("Wflat", [D, L * 3 * D], f32, isOutput=False)
    b_in = nc.declare_dram_parameter("bvec", [D, L], f32, isOutput=False)
    pw_in = nc.declare_dram_parameter("pwv", [D, 1], f32, isOutput=False)
    pb_in = nc.declare_dram_parameter("pbv", [1, 1], f32, isOutput=False)
    y_out = nc.declare_dram_parameter("y", [NSH, 1], f32, isOutput=True)

    ag = {}
    for nm in ("u1", "h1", "u1b"):
        ag[nm] = (
            nc.dram_tensor(f"agin_{nm}", [ROWS_PAD, D], f32),
            nc.dram_tensor(f"agout_{nm}", [TBL_ROWS, D], f32, addr_space="Shared"),
        )

    with tile.TileContext(nc) as tc:
        with (
            tc.tile_pool(name="const", bufs=1) as cp,
            tc.tile_pool(name="idxp", bufs=4) as ip,
            tc.tile_pool(name="gbuf", bufs=2) as gp,
            tc.tile_pool(name="mbuf", bufs=2) as mp,
            tc.tile_pool(name="sT", bufs=1) as sp,
            tc.tile_pool(name="rows", bufs=1) as rp,
            tc.tile_pool(name="spsum", bufs=2, space="PSUM") as pp,
            tc.tile_pool(name="opsum", bufs=2, space="PSUM") as tp,
        ):
            dcol = cp.tile([128, NTILE_TOT], f32)
            nc.sync.dma_start(out=dcol[:], in_=dcol_in[:])
            wval = cp.tile([128, NTILE_TOT], f32)
            nc.sync.dma_start(out=wval[:], in_=wval_in[:])
            iota = cp.tile([128, WIN], f32)
            nc.sync.dma_start(out=iota[:], in_=iota_in[:])
            wfl = cp.tile([D, L * 3 * D], f32)
            nc.sync.dma_start(out=wfl[:], in_=W_in[:])
            bv = cp.tile([D, L], f32)
            nc.sync.dma_start(out=bv[:], in_=b_in[:])
            pwv = cp.tile([D, 1], f32)
            nc.sync.dma_start(out=pwv[:], in_=pw_in[:])
            pbv = cp.tile([1, 1], f32)
            nc.sync.dma_start(out=pbv[:], in_=pb_in[:])
            f0T = cp.tile([D, NCOLS], f32)
            nc.sync.dma_start(out=f0T[:], in_=f0T_in[:])
            ident = cp.tile([128, 128], f32)
            make_identity(nc, ident[:])

            u1T = sp.tile([D, NCOLS], f32, tag="u1T")
            if mode == "full":
                h1T = sp.tile([D, NCOLS], f32, tag="h1T")

            gq = [0]

            def spmm_chunk(table, c, tag):
                """Gathers + mask gen + PE reduce for chunk c. Returns psum
                tile [64, CCOLS] (caller evacuates / consumes)."""
                idxc = ip.tile([128, SLOTS_CH // 16], mybir.dt.int16, tag="idxc",
                               name=f"idxc_{tag}_{c}")
                nc.sync.dma_start(
                    out=idxc[:],
                    in_=idx_in[:, c * (SLOTS_CH // 16):(c + 1) * (SLOTS_CH // 16)])
                # one tile per dma_gather so the 12 gathers of a chunk have no
                # same-tile WAW ordering and pipeline across all 4 SWDGE queues
                gsub = {}
                for h in (0, 1):
                    tab = table[h * HALF:(h + 1) * HALF, :]
                    for g in range(GPC):
                        gt = gp.tile([128, GIDX // 128, D], f32, tag=f"g{h}{g}",
                                     name=f"g{h}{g}_{tag}_{c}")
                        gsub[(h, g)] = gt
                        off = (h * (SLOTS_CH // 2) + g * GIDX) // 16
                        nc.gpsimd.dma_gather(
                            gt[:],
                            tab,
                            idxc[:, off:off + GIDX // 16],
                            GIDX, GIDX, D,
                            queue_num=gq[0] % 4,
                        )
                        gq[0] += 1
                mask = mp.tile([128, NTILE_CH * WIN], f32, tag="mask",
                               name=f"mask_{tag}_{c}")
                m3 = mask[:].rearrange("p (t o) -> p t o", o=WIN)
                i3 = iota[:].rearrange("p (o t) -> p o t", o=1).to_broadcast(
                    [128, NTILE_CH, WIN])
                d3 = dcol[:, c * NTILE_CH:(c + 1) * NTILE_CH].rearrange(
                    "p (t o) -> p t o", o=1).to_broadcast([128, NTILE_CH, WIN])
                w3 = wval[:, c * NTILE_CH:(c + 1) * NTILE_CH].rearrange(
                    "p (t o) -> p t o", o=1).to_broadcast([128, NTILE_CH, WIN])
                nc.vector.tensor_tensor(out=m3, in0=i3, in1=d3,
                                        op=mybir.AluOpType.is_equal)
                nc.vector.tensor_tensor(out=m3, in0=m3, in1=w3,
                                        op=mybir.AluOpType.mult)
                ps = pp.tile([64, CCOLS], f32, tag="spsum", name=f"ps_{tag}_{c}")
                TN = GIDX // 128
                for w in range(CWIN):
                    for j in range(2 * TPW):
                        h = 0 if j < TPW else 1
                        t = TPW * w + (j % TPW)
                        lhsT = gsub[(h, t // TN)][:, t % TN, :]
                        mt = w * (2 * TPW) + j
                        nc.tensor.matmul(
                            ps[:, WIN * w:WIN * (w + 1)],
                            lhsT,
                            mask[:, mt * WIN:(mt + 1) * WIN],
                            start=(j == 0), stop=(j == 2 * TPW - 1),
                        )
                return ps

            def spmm(table, out_sT, tag):
                for c in range(NCH):
                    ps = spmm_chunk(table, c, tag)
                    nc.vector.tensor_copy(
                        out=out_sT[:, c * CCOLS:(c + 1) * CCOLS], in_=ps[:])

            def spmm_fused_dense(table, l, u0T, u1T_, outT, tag):
                """SpMM for U2 fused with the dense layer + (layer 2) head."""
                for c in range(NCH):
                    ps = spmm_chunk(table, c, tag)
                    u2c = mp.tile([64, CCOLS], f32, tag="u2c", name=f"u2c_{tag}_{c}")
                    nc.vector.tensor_copy(out=u2c[:], in_=ps[:])
                    dp = tp.tile([64, CCOLS], f32, tag="dpsum", name=f"dp_{tag}_{c}")
                    for t, uT in enumerate((u0T, u1T_, u2c)):
                        rhs = uT[:] if t == 2 else uT[:, c * CCOLS:(c + 1) * CCOLS]
                        nc.tensor.matmul(
                            dp[:],
                            wfl[:, (l * 3 + t) * D:(l * 3 + t + 1) * D],
                            rhs,
                            start=(t == 0), stop=(t == 2),
                        )
                    nc.scalar.activation(
                        out=outT[:, c * CCOLS:(c + 1) * CCOLS],
                        in_=dp[:],
                        func=mybir.ActivationFunctionType.Relu,
                        bias=bv[:, l:l + 1],
                        scale=1.0,
                    )
                    if l == L - 1 and c * CCOLS < NSH:
                        hp = tp.tile([1, CCOLS], f32, tag="hpsum", name=f"hp_{c}")
                        nc.tensor.matmul(
                            hp[:], pwv[:],
                            outT[:, c * CCOLS:(c + 1) * CCOLS],
                            start=True, stop=True)
                        nv = min(NSH, (c + 1) * CCOLS) - c * CCOLS
                        yc = ip.tile([1, CCOLS], f32, tag="yc", name=f"yc_{c}")
                        nc.vector.tensor_scalar(
                            out=yc[:1, :], in0=hp[:], scalar1=pbv[:1, :1],
                            scalar2=None, op0=mybir.AluOpType.add)
                        nc.sync.dma_start(
                            out=y_out[c * CCOLS:c * CCOLS + nv, :],
                            in_=yc[:1, :nv])

            def transpose_out(sT, agin, tag):
                rows = rp.tile([128, NCOLS // 128, D], f32, tag="rows",
                               name=f"rows_{tag}")
                for k in range(NCOLS // 128):
                    tps = tp.tile([128, D], f32, tag="tpsum", name=f"tps_{tag}_{k}")
                    nc.tensor.transpose(tps[:], sT[:, k * 128:(k + 1) * 128],
                                        ident[:64, :64])
                    nc.vector.tensor_copy(out=rows[:, k, :], in_=tps[:])
                nc.sync.dma_start(
                    out=agin.ap().rearrange("(k p) d -> p k d", p=128), in_=rows[:])

            def allgather(nm):
                agin, agout = ag[nm]
                nc.gpsimd.collective_compute(
                    "AllGather",
                    mybir.AluOpType.bypass,
                    ins=[agin.ap().opt()],
                    outs=[agout.ap().opt()],
                    replica_groups=[list(range(NCORES))],
                )

            if mode == "full":
                h2T = sp.tile([D, NCOLS], f32, tag="h2T")
            if mode == "spmm_only":
                for r in range(repeat):
                    spmm(feat_pad, u1T, f"r{r}s1")
                nc.sync.dma_start(out=y_out[:, :], in_=u1T[:1, :NSH])
            elif mode == "gather_only":
                # gathers + idx DMAs only; consume via tiny copy to keep deps
                for r in range(repeat):
                    for c in range(NCH):
                        idxc = ip.tile([128, SLOTS_CH // 16], mybir.dt.int16,
                                       tag="idxc", name=f"gi_{r}_{c}")
                        nc.sync.dma_start(
                            out=idxc[:],
                            in_=idx_in[:, c * (SLOTS_CH // 16):(c + 1) * (SLOTS_CH // 16)])
                        glo = gp.tile([128, CWIN * TPW, D], f32, tag="glo",
                                      name=f"gg_{r}_{c}")
                        ghi = gp.tile([128, CWIN * TPW, D], f32, tag="ghi",
                                      name=f"gh_{r}_{c}")
                        for h, gbuf in ((0, glo), (1, ghi)):
                            tab = feat_pad[h * HALF:(h + 1) * HALF, :]
                            for g in range(GPC):
                                off = (h * (SLOTS_CH // 2) + g * GIDX) // 16
                                nc.gpsimd.dma_gather(
                                    gbuf[:, g * (GIDX // 128):(g + 1) * (GIDX // 128), :],
                                    tab, idxc[:, off:off + GIDX // 16],
                                    GIDX, GIDX, D, queue_num=gq[0] % 4)
                                gq[0] += 1
                        nc.vector.tensor_copy(out=u1T[:1, c * 8:(c + 1) * 8],
                                              in_=glo[:1, 0, :8])
                nc.sync.dma_start(out=y_out[:, :], in_=u1T[:1, :NSH])
            elif mode == "ag_only":
                for r in range(repeat):
                    transpose_out(f0T, ag["u1"][0], f"r{r}t1")
                    allgather("u1")
                nc.sync.dma_start(out=y_out[:, :], in_=f0T[:1, :NSH])
            elif mode == "tr_only":
                for r in range(repeat):
                    transpose_out(f0T, ag["u1"][0], f"r{r}t1")
                nc.sync.dma_start(out=y_out[:, :], in_=f0T[:1, :NSH])
            if mode != "full":
                repeat = 0
            for r in range(repeat):
                # ---- layer 1 ----
                spmm(feat_pad, u1T, f"r{r}s1")
                transpose_out(u1T, ag["u1"][0], f"r{r}t1")
                allgather("u1")
                # h1T <- relu(f0 Wa + u1 Wb + u2 Wc + b0), u2 fused from SpMM2
                spmm_fused_dense(ag["u1"][1], 0, f0T, u1T, h1T, f"r{r}s2")
                transpose_out(h1T, ag["h1"][0], f"r{r}t2")
                allgather("h1")
                # ---- layer 2 ----
                spmm(ag["h1"][1], u1T, f"r{r}s3")
                transpose_out(u1T, ag["u1b"][0], f"r{r}t3")
                allgather("u1b")
                spmm_fused_dense(ag["u1b"][1], 1, h1T, u1T, h2T, f"r{r}s4")


    _finalize_with_split(nc)
    return nc


def _get_runner():
    if "runner" in _CACHE:
        return _CACHE["runner"]
    nc = _build_nc()
    _CACHE["runner"] = _build_runner(nc, NCORES)
    return _CACHE["runner"]


def kernel(features, src, dst, W, b, pw, pb):
    in_maps = _host_prep(features, src, dst, W, b, pw, pb)
    run = _get_runner()
    results, times = run(in_maps, iters=1)
    _CACHE["last_times"] = times
    y = np.concatenate([results[i]["y"] for i in range(NCORES)], axis=0)
    return y.astype(np.float32)



# revision 17
# speedup vs baseline: 1.9987x; 1.9987x over previous
"""ChebNet (K=3, L=2) forward on 8 Trainium2 NeuronCores.

Node-sharded SPMD, compiled per graph instance. Each core owns 6250 dst rows,
LPT-permuted into 104 windows of 64 columns so window edge counts are nearly
equal; the shared program uses max-over-cores tile counts per (chunk, half,
window) group (~10% slot padding vs 60% for a fixed layout).

SpMM = one dma_gather per (chunk, table-half) (26 calls/SpMM vs 312) of f32
feature rows + PE matmuls against on-device 0/1 masks (single is_equal; no
weight multiply). norm[src] is folded into the gather tables (host prescale /
transpose-evac row scale), norm[dst] into a per-column multiply of the dense
accumulation, which commutes through the weight matmul:

  P1 = A(Nx); T1 = -N P1; table2 = -N^2 P1; P2 = A table2; T2 = -2N P2 - T0
  out^T = (W0-W2)^T T0^T + N o [(-W1)^T P1^T + (-2 W2)^T P2^T]; h = relu(out+b)

Full node tables are rebuilt between SpMMs with AllGather (3 per pass).
"""
import math
import numpy as np

N, E, D, K, L = 50000, 800000, 64, 3, 2
NCORES = 8
NSH = N // NCORES              # 6250 dst rows per core
MWIN = 64                      # mask window: dst cols per mask tile
CHW = 8                        # windows per chunk
CCOLS = MWIN * CHW             # 512 cols per chunk (one PSUM bank)
NCH = 13                       # chunks per core
NCOLS = NCH * CCOLS            # 6656 padded cols per core
NW = NCH * CHW                 # 104 windows per core
ROWS_PAD = NCOLS               # padded rows per core block in the table
TBL_ROWS = NCORES * ROWS_PAD   # 53248
HALF = TBL_ROWS // 2           # 26624 (< 32768: int16-reachable)
GMAX_T = 8                     # max 128-slot tiles per dma_gather call

_CACHE = {}


# ---------------------------------------------------------------------------
# Workaround for this walrus build: any instruction carrying >1 sync wait is
# rejected ("Too many sync wait commands"). Hoist extras onto 1-wait NoOps on
# the same engine (per-engine program order preserves semantics).
_ws_counter = [0]


def _split_multiwaits(nc):
    import concourse.mybir as mybir
    n_split = 0
    for fn in nc.m.functions:
        for bb in fn.blocks:
            new_list = []
            changed = False
            for inst in bb.instructions:
                si = inst.sync_info
                waits = list(si.on_wait) if si is not None else []
                if len(waits) > 1:
                    changed = True
                    for w in waits[:-1]:
                        _ws_counter[0] += 1
                        nop = mybir.InstNoOp(
                            name=f"waitsplit-{_ws_counter[0]}",
                            ins=[], outs=[],
                            sync_info=mybir.SyncInfo(on_wait=[w], on_update=[]),
                        )
                        nop.engine = inst.engine
                        nc.register_instruction(nop, overwrite=True)
                        new_list.append(nop)
                        n_split += 1
                    si.on_wait = waits[-1:]
                new_list.append(inst)
            if changed:
                bb.instructions[:] = new_list
    return n_split


def _finalize_with_split(nc):
    import concourse.bass as _bass
    nc.compile()           # Bacc passes (incl. library-load insertion)
    _split_multiwaits(nc)  # after replace_nops_with_events, before freeze
    _bass.Bass.finalize(nc)


def _build_runner(nc, n_cores):
    """SPMD runner over the axon PJRT backend (keeps the jitted executable
    and device-resident inputs so repeat calls can be timed)."""
    import jax
    from jax.sharding import Mesh, PartitionSpec
    from jax.experimental.shard_map import shard_map
    import concourse.mybir as mybir
    from concourse.bass2jax import (
        _bass_exec_p, install_neuronx_cc_hook, partition_id_tensor)

    install_neuronx_cc_hook()
    partition_name = nc.partition_id_tensor.name if nc.partition_id_tensor else None

    in_names, out_names, out_avals, zero_outs = [], [], [], []
    for alloc in nc.m.functions[0].allocations:
        if not isinstance(alloc, mybir.MemoryLocationSet):
            continue
        name = alloc.memorylocations[0].name
        if alloc.kind == "ExternalInput":
            if name != partition_name:
                in_names.append(name)
        elif alloc.kind == "ExternalOutput":
            shape = tuple(alloc.tensor_shape)
            dtype = mybir.dt.np(alloc.dtype)
            out_names.append(name)
            out_avals.append(jax.core.ShapedArray(shape, dtype))
            zero_outs.append(np.zeros(shape, dtype))
    n_params = len(in_names)
    all_in_names = list(in_names) + list(out_names)
    if partition_name is not None:
        all_in_names.append(partition_name)

    def _body(*args):
        operands = list(args)
        if partition_name is not None:
            operands.append(partition_id_tensor())
        outs = _bass_exec_p.bind(
            *operands,
            out_avals=tuple(out_avals),
            in_names=tuple(all_in_names),
            out_names=tuple(out_names),
            lowering_input_output_aliases=(),
            sim_require_finite=True,
            sim_require_nnan=True,
            nc=nc,
        )
        return tuple(outs)

    devices = jax.devices()[:n_cores]
    mesh = Mesh(np.asarray(devices), ("core",))
    in_specs = (PartitionSpec("core"),) * (n_params + len(out_names))
    out_specs = (PartitionSpec("core"),) * len(out_names)
    sharded = jax.jit(
        shard_map(_body, mesh=mesh, in_specs=in_specs, out_specs=out_specs,
                  check_rep=False),
        keep_unused=True,
    )

    def run(in_maps, iters=1):
        import time as _time
        per_core = [[np.asarray(m[name]) for name in in_names] for m in in_maps]
        concat_in = [
            np.concatenate([per_core[c][i] for c in range(n_cores)], axis=0)
            for i in range(n_params)
        ]
        concat_zeros = [
            np.zeros((n_cores * z.shape[0], *z.shape[1:]), z.dtype)
            for z in zero_outs
        ]
        sharding = jax.sharding.NamedSharding(mesh, PartitionSpec("core"))
        dev_in = [jax.device_put(a, sharding) for a in concat_in + concat_zeros]
        out = sharded(*dev_in)
        jax.block_until_ready(out)
        times = []
        for _ in range(iters):
            t0 = _time.perf_counter()
            out = sharded(*dev_in)
            jax.block_until_ready(out)
            times.append(_time.perf_counter() - t0)
        results = [
            {name: np.asarray(out[i]).reshape(n_cores, *out_avals[i].shape)[c]
             for i, name in enumerate(out_names)}
            for c in range(n_cores)
        ]
        return results, times

    return run


def _lpt_windows(deg_local):
    """Assign each of the core's nodes to one of NW windows (<=64 nodes each)
    balancing total degree (LPT greedy). Returns pos[node] in [0, NCOLS)."""
    import heapq
    n = deg_local.shape[0]
    order = np.argsort(-deg_local, kind="stable")
    heap = [(0.0, w) for w in range(NW)]
    heapq.heapify(heap)
    counts = np.zeros(NW, dtype=np.int64)
    fill = np.zeros(NW, dtype=np.int64)
    pos = np.empty(n, dtype=np.int64)
    spill = []
    for node in order:
        s, w = heapq.heappop(heap)
        pos[node] = (w // CHW) * CCOLS + (w % CHW) * MWIN + counts[w]
        counts[w] += 1
        s += deg_local[node]
        if counts[w] < MWIN:
            heapq.heappush(heap, (s, w))
    return pos


def _host_prep(features, src, dst, W, b, pw, pb):
    src = np.asarray(src).astype(np.int64)
    dst = np.asarray(dst).astype(np.int64)
    features = np.asarray(features, dtype=np.float32)
    W = np.asarray(W, dtype=np.float32)
    b = np.asarray(b, dtype=np.float32)
    pw = np.asarray(pw, dtype=np.float32).reshape(D, 1)
    pb = np.asarray(pb, dtype=np.float32).reshape(1)

    deg = np.bincount(dst, minlength=N).astype(np.float32)
    norm = np.clip(deg, 1.0, None) ** -0.5

    # total degree (in+out) balance proxy: edges where node appears as src
    # drive gather load; edges as dst drive window load. Windows hold dst
    # rows, so balance on in-degree.
    core_of_dst = dst // NSH
    pos_local = np.empty(N, dtype=np.int64)   # permuted col within core
    for i in range(NCORES):
        nodes = np.arange(i * NSH, (i + 1) * NSH)
        pos_local[nodes] = _lpt_windows(deg[nodes])
    gpos = (np.arange(N) // NSH) * ROWS_PAD + pos_local  # global table row

    # per-edge quantities
    e_core = core_of_dst
    e_pos = pos_local[dst]                   # dst col within owner core
    e_win = e_pos // MWIN                    # global window 0..103
    e_dcol = (e_pos % MWIN).astype(np.float32)
    e_srow = gpos[src]                       # src row in padded table
    e_half = (e_srow >= HALF).astype(np.int64)

    # group counts per (core, win, half) -> shared tile counts k = max
    gk = e_core * (NW * 2) + e_win * 2 + e_half
    cnt = np.bincount(gk, minlength=NCORES * NW * 2).reshape(NCORES, NW, 2)
    kk = -(-cnt.max(axis=0) // 128)          # [NW, 2] tiles per (win, half)
    kk = np.maximum(kk, 1)

    # slot/tile layout shared by all cores.
    # chunk c: call h=0 -> windows 0..7 tiles, then call h=1.
    kc = kk.reshape(NCH, CHW, 2)
    call_tiles = kc.sum(axis=1)              # [NCH, 2] tiles per call
    call_slots = call_tiles * 128
    ntile_chunk = call_tiles.sum(axis=1)     # [NCH]
    NTILE_TOT = int(ntile_chunk.sum())
    SLOTS_TOT = NTILE_TOT * 128
    # tile base within chunk for (h, w)
    tile_base = np.zeros((NCH, 2, CHW), dtype=np.int64)
    for c in range(NCH):
        t = 0
        for h in (0, 1):
            for w in range(CHW):
                tile_base[c, h, w] = t
                t += kc[c, w, h]
    chunk_tile_off = np.concatenate([[0], np.cumsum(ntile_chunk)])[:-1]
    chunk_slot_off = chunk_tile_off * 128
    call_slot_off = np.zeros((NCH, 2), dtype=np.int64)
    for c in range(NCH):
        call_slot_off[c, 0] = chunk_slot_off[c]
        call_slot_off[c, 1] = chunk_slot_off[c] + call_slots[c, 0]

    meta = {
        "kc": kc, "call_tiles": call_tiles, "call_slots": call_slots,
        "ntile_chunk": ntile_chunk, "NTILE_TOT": NTILE_TOT,
        "SLOTS_TOT": SLOTS_TOT, "tile_base": tile_base,
        "chunk_tile_off": chunk_tile_off,
    }

    # shared scaled feature table: row gpos[g] = norm[g] * x[g]
    feat_scaled = np.zeros((TBL_ROWS, D), dtype=np.float32)
    feat_scaled[gpos] = features * norm[:, None]

    Wflat = np.zeros((D, L * 3 * D), dtype=np.float32)
    for l in range(L):
        for t, Wt in enumerate((W[l, 0] - W[l, 2], -W[l, 1], -2.0 * W[l, 2])):
            Wflat[:, (l * 3 + t) * D:(l * 3 + t + 1) * D] = Wt

    iota = np.tile(np.arange(MWIN, dtype=np.float32)[None, :], (128, 1))

    in_maps = []
    perms = []
    for i in range(NCORES):
        sel = e_core == i
        s_srow = e_srow[sel]
        s_half = e_half[sel]
        s_win = e_win[sel]
        s_dcol = e_dcol[sel]

        # rank within (win, half) group
        key = s_win * 2 + s_half
        order = np.argsort(key, kind="stable")
        ks = key[order]
        grp_start = np.searchsorted(ks, np.arange(NW * 2), side="left")
        rank = np.arange(ks.size) - grp_start[ks]

        ww = ks // 2
        hh = ks % 2
        c_ = ww // CHW
        wl = ww % CHW
        mt = chunk_tile_off[c_] + tile_base[c_, hh, wl] + rank // 128
        slot = mt * 128 + rank % 128

        idx_slots = np.zeros(SLOTS_TOT, dtype=np.int16)
        dcol_slots = np.full(SLOTS_TOT, -1.0, dtype=np.float32)
        idx_slots[slot] = (s_srow[order] - hh * HALF).astype(np.int16)
        dcol_slots[slot] = s_dcol[order]

        # idx wrap per call: slot i of call -> [i%16, i//16]
        idx_arr = np.zeros((16, SLOTS_TOT // 16), dtype=np.int16)
        off16 = 0
        for c in range(NCH):
            for h in (0, 1):
                s0 = call_slot_off[c, h]
                n_ = call_slots[c, h]
                wrap = idx_slots[s0:s0 + n_].reshape(n_ // 16, 16).T
                idx_arr[:, off16:off16 + n_ // 16] = wrap
                off16 += n_ // 16
        idx_arr = np.tile(idx_arr, (8, 1))

        dcol = dcol_slots.reshape(NTILE_TOT, 128).T.copy()

        # local node data in permuted order
        pos = pos_local[i * NSH:(i + 1) * NSH]
        perms.append(pos)
        nloc = norm[i * NSH:(i + 1) * NSH]
        f0T = np.zeros((D, NCOLS), dtype=np.float32)
        f0T[:, pos] = features[i * NSH:(i + 1) * NSH].T
        normB_row = np.zeros(NCOLS, dtype=np.float32)
        normB_row[pos] = nloc
        normB = np.tile(normB_row[None, :], (D, 1))
        nsq_m2 = np.zeros((128, NCOLS // 128), dtype=np.float32)
        nsq_p1 = np.zeros((128, NCOLS // 128), dtype=np.float32)
        nrm_cols = np.zeros(NCOLS, dtype=np.float32)
        nrm_cols[pos] = nloc
        nsq_m2[:, :] = (-(nrm_cols ** 2)).reshape(NCOLS // 128, 128).T
        nsq_p1[:, :] = nrm_cols.reshape(NCOLS // 128, 128).T

        in_maps.append({
            "feat_tbl": feat_scaled,
            "f0T": f0T,
            "idx_all": idx_arr,
            "dcol": dcol,
            "iota": iota,
            "normB": normB,
            "nsq_m2": nsq_m2,
            "nsq_p1": nsq_p1,
            "Wflat": Wflat,
            "bvec": b.T.copy(),
            "pwv": pw,
            "pbv": pb.reshape(1, 1),
        })
    return in_maps, meta, perms


def _build_nc(meta, repeat=1, mode="full"):
    import concourse.bacc as bacc
    import concourse.mybir as mybir
    import concourse.tile as tile
    from concourse.masks import make_identity
    f32 = mybir.dt.float32

    kc = meta["kc"]
    call_tiles = meta["call_tiles"]
    call_slots = meta["call_slots"]
    ntile_chunk = meta["ntile_chunk"]
    NTILE_TOT = meta["NTILE_TOT"]
    SLOTS_TOT = meta["SLOTS_TOT"]
    tile_base = meta["tile_base"]
    chunk_tile_off = meta["chunk_tile_off"]

    nc = bacc.Bacc("TRN2", num_swdge_queues=4)
    feat_tbl = nc.declare_dram_parameter("feat_tbl", [TBL_ROWS, D], f32, isOutput=False)
    f0T_in = nc.declare_dram_parameter("f0T", [D, NCOLS], f32, isOutput=False)
    idx_in = nc.declare_dram_parameter("idx_all", [128, SLOTS_TOT // 16], mybir.dt.int16, isOutput=False)
    dcol_in = nc.declare_dram_parameter("dcol", [128, NTILE_TOT], f32, isOutput=False)
    iota_in = nc.declare_dram_parameter("iota", [128, MWIN], f32, isOutput=False)
    normB_in = nc.declare_dram_parameter("normB", [D, NCOLS], f32, isOutput=False)
    nsqm2_in = nc.declare_dram_parameter("nsq_m2", [128, NCOLS // 128], f32, isOutput=False)
    nsqp1_in = nc.declare_dram_parameter("nsq_p1", [128, NCOLS // 128], f32, isOutput=False)
    W_in = nc.declare_dram_parameter("Wflat", [D, L * 3 * D], f32, isOutput=False)
    b_in = nc.declare_dram_parameter("bvec", [D, L], f32, isOutput=False)
    pw_in = nc.declare_dram_parameter("pwv", [D, 1], f32, isOutput=False)
    pb_in = nc.declare_dram_parameter("pbv", [1, 1], f32, isOutput=False)
    y_out = nc.declare_dram_parameter("y", [NCOLS, 1], f32, isOutput=True)

    ag = {}
    for nm in ("t2", "t3", "t4"):
        ag[nm] = (
            nc.dram_tensor(f"agin_{nm}", [ROWS_PAD, D], f32),
            nc.dram_tensor(f"agout_{nm}", [TBL_ROWS, D], f32, addr_space="Shared"),
        )

    with tile.TileContext(nc) as tc:
        with (
            tc.tile_pool(name="const", bufs=1) as cp,
            tc.tile_pool(name="idxp", bufs=2) as ip,
            tc.tile_pool(name="gbuf", bufs=2) as gp,
            tc.tile_pool(name="mbuf", bufs=2) as mp,
            tc.tile_pool(name="sT", bufs=1) as sp,
            tc.tile_pool(name="rows", bufs=1) as rp,
            tc.tile_pool(name="small", bufs=2) as qp,
            tc.tile_pool(name="spsum", bufs=2, space="PSUM") as pp,
            tc.tile_pool(name="dpsum", bufs=1, space="PSUM") as dp_pool,
            tc.tile_pool(name="tpsum", bufs=2, space="PSUM") as tp,
        ):
            dcol = cp.tile([128, NTILE_TOT], f32)
            nc.sync.dma_start(out=dcol[:], in_=dcol_in[:])
            iota = cp.tile([128, MWIN], f32)
            nc.sync.dma_start(out=iota[:], in_=iota_in[:])
            wfl = cp.tile([D, L * 3 * D], f32)
            nc.sync.dma_start(out=wfl[:], in_=W_in[:])
            bv = cp.tile([D, L], f32)
            nc.sync.dma_start(out=bv[:], in_=b_in[:])
            pwv = cp.tile([D, 1], f32)
            nc.sync.dma_start(out=pwv[:], in_=pw_in[:])
            pbv = cp.tile([1, 1], f32)
            nc.sync.dma_start(out=pbv[:], in_=pb_in[:])
            normB = cp.tile([D, NCOLS], f32)
            nc.sync.dma_start(out=normB[:], in_=normB_in[:])
            nsqm2 = cp.tile([128, NCOLS // 128], f32)
            nc.sync.dma_start(out=nsqm2[:], in_=nsqm2_in[:])
            nsqp1 = cp.tile([128, NCOLS // 128], f32)
            nc.sync.dma_start(out=nsqp1[:], in_=nsqp1_in[:])
            ident = cp.tile([64, 64], f32)
            make_identity(nc, ident[:])

            p1T = sp.tile([D, NCOLS], f32, tag="p1T")
            if mode in ("full", "noag"):
                h1T = sp.tile([D, NCOLS], f32, tag="h1T")

            def spmm_chunk(table, c, tag):
                """Gathers + mask gen + PE reduce for chunk c. Returns psum
                tile [64, CCOLS] (caller evacuates / consumes)."""
                ntc = int(ntile_chunk[c])
                i16_off = int(chunk_tile_off[c]) * 8  # slots/16
                idxc = ip.tile([128, ntc * 8], mybir.dt.int16, tag="idxc",
                               name=f"idxc_{tag}_{c}")
                nc.sync.dma_start(
                    out=idxc[:], in_=idx_in[:, i16_off:i16_off + ntc * 8])
                gsub = {}
                for h in (0, 1):
                    kt = int(call_tiles[c, h])
                    tab = table[h * HALF:(h + 1) * HALF, :]
                    coff = 0 if h == 0 else int(call_slots[c, 0]) // 16
                    for p in range(-(-kt // GMAX_T)):
                        t0 = p * GMAX_T
                        tn = min(GMAX_T, kt - t0)
                        gt = gp.tile([128, tn, D], f32, tag=f"g{h}p{p}",
                                     name=f"g{h}p{p}_{tag}_{c}")
                        gsub[(h, p)] = gt
                        ioff = coff + t0 * 8
                        nc.gpsimd.dma_gather(
                            gt[:],
                            tab,
                            idxc[:, ioff:ioff + tn * 8],
                            tn * 128, tn * 128, D,
                            queue_num=(2 * h + c % 2 + p) % 4,
                        )
                mask = mp.tile([128, ntc * MWIN], f32, tag="mask",
                               name=f"mask_{tag}_{c}")
                m3 = mask[:].rearrange("p (t o) -> p t o", o=MWIN)
                i3 = iota[:].rearrange("p (o t) -> p o t", o=1).to_broadcast(
                    [128, ntc, MWIN])
                d3 = dcol[:, chunk_tile_off[c]:chunk_tile_off[c] + ntc].rearrange(
                    "p (t o) -> p t o", o=1).to_broadcast([128, ntc, MWIN])
                nc.vector.tensor_tensor(out=m3, in0=i3, in1=d3,
                                        op=mybir.AluOpType.is_equal)
                ps = pp.tile([64, CCOLS], f32, tag="spsum", name=f"ps_{tag}_{c}")
                for w in range(CHW):
                    runs = []
                    for h in (0, 1):
                        base = int(tile_base[c, h, w])
                        for j in range(int(kc[c, w, h])):
                            runs.append((h, base + j))
                    for r, (h, tglob) in enumerate(runs):
                        tcall = tglob - (int(call_tiles[c, 0]) if h else 0)
                        nc.tensor.matmul(
                            ps[:, MWIN * w:MWIN * (w + 1)],
                            gsub[(h, tcall // GMAX_T)][:, tcall % GMAX_T, :],
                            mask[:, tglob * MWIN:(tglob + 1) * MWIN],
                            start=(r == 0), stop=(r == len(runs) - 1),
                        )
                return ps

            def spmm(table, out_sT, tag, scale, agin):
                """SpMM writing transposed result to out_sT and (scaled)
                row-major rows to the allgather input buffer."""
                rows = rp.tile([128, NCOLS // 128, D], f32, tag="rows",
                               name=f"rows_{tag}")
                for c in range(NCH):
                    ps = spmm_chunk(table, c, tag)
                    nc.vector.tensor_copy(
                        out=out_sT[:, c * CCOLS:(c + 1) * CCOLS], in_=ps[:])
                    for kb in range(CCOLS // 128):
                        k = c * (CCOLS // 128) + kb
                        tps = tp.tile([128, D], f32, tag="tpsum",
                                      name=f"tps_{tag}_{k}")
                        nc.tensor.transpose(
                            tps[:], out_sT[:, k * 128:(k + 1) * 128], ident[:])
                        nc.vector.tensor_scalar(
                            out=rows[:, k, :], in0=tps[:],
                            scalar1=scale[:, k:k + 1], scalar2=None,
                            op0=mybir.AluOpType.mult)
                nc.sync.dma_start(
                    out=agin.ap().rearrange("(k p) d -> p k d", p=128),
                    in_=rows[:])

            def spmm_fused_dense(table, l, t0T, p1T_, outT, tag,
                                 agin=None, scale=None, head=False):
                """SpMM for P2 fused with the dense layer; optionally also
                writes scale*h rows to agin (layer-1 h -> table3), and the
                prediction head (layer 2)."""
                if agin is not None:
                    rows = rp.tile([128, NCOLS // 128, D], f32, tag="rows",
                                   name=f"rows_{tag}")
                for c in range(NCH):
                    ps = spmm_chunk(table, c, tag)
                    cc = slice(c * CCOLS, (c + 1) * CCOLS)
                    if outT is None:
                        hout = qp.tile([64, CCOLS], f32, tag="h2c",
                                       name=f"h2c_{tag}_{c}")
                        occ = slice(0, CCOLS)
                    else:
                        hout = outT
                        occ = cc
                    p2c = qp.tile([64, CCOLS], f32, tag="p2c", name=f"p2c_{tag}_{c}")
                    nc.vector.tensor_copy(out=p2c[:], in_=ps[:])
                    # scaled group: W1' P1 + W2' P2
                    dps = dp_pool.tile([64, CCOLS], f32, tag="dps",
                                       name=f"dps_{tag}_{c}")
                    nc.tensor.matmul(
                        dps[:], wfl[:, (l * 3 + 1) * D:(l * 3 + 2) * D],
                        p1T_[:, cc], start=True, stop=False)
                    nc.tensor.matmul(
                        dps[:], wfl[:, (l * 3 + 2) * D:(l * 3 + 3) * D],
                        p2c[:], start=False, stop=True)
                    # plain group: (W0-W2) T0
                    if t0T is None:   # layer 1: stream x^T chunk from DRAM
                        t0c = qp.tile([64, CCOLS], f32, tag="t0c",
                                      name=f"t0c_{tag}_{c}")
                        nc.sync.dma_start(out=t0c[:], in_=f0T_in[:, cc])
                        t0v = t0c[:]
                    else:
                        t0v = t0T[:, cc]
                    dpp = dp_pool.tile([64, CCOLS], f32, tag="dpp",
                                       name=f"dpp_{tag}_{c}")
                    nc.tensor.matmul(
                        dpp[:], wfl[:, (l * 3 + 0) * D:(l * 3 + 1) * D],
                        t0v, start=True, stop=True)
                    tmp = qp.tile([64, CCOLS], f32, tag="tmp", name=f"tmp_{tag}_{c}")
                    nc.vector.tensor_tensor(out=tmp[:], in0=dps[:],
                                            in1=normB[:, cc],
                                            op=mybir.AluOpType.mult)
                    hpre = qp.tile([64, CCOLS], f32, tag="hpre",
                                   name=f"hpre_{tag}_{c}")
                    nc.vector.tensor_tensor(out=hpre[:], in0=tmp[:], in1=dpp[:],
                                            op=mybir.AluOpType.add)
                    nc.scalar.activation(
                        out=hout[:, occ], in_=hpre[:],
                        func=mybir.ActivationFunctionType.Relu,
                        bias=bv[:, l:l + 1], scale=1.0)
                    if agin is not None:
                        for kb in range(CCOLS // 128):
                            k = c * (CCOLS // 128) + kb
                            tps = tp.tile([128, D], f32, tag="tpsum",
                                          name=f"tps_{tag}_{k}")
                            nc.tensor.transpose(
                                tps[:],
                                outT[:, c * CCOLS + kb * 128:
                                     c * CCOLS + (kb + 1) * 128],
                                ident[:])
                            nc.vector.tensor_scalar(
                                out=rows[:, k, :], in0=tps[:],
                                scalar1=scale[:, k:k + 1], scalar2=None,
                                op0=mybir.AluOpType.mult)
                    if head:
                        hp = tp.tile([1, CCOLS], f32, tag="hpsum",
                                     name=f"hp_{c}")
                        nc.tensor.matmul(hp[:], pwv[:], hout[:, occ],
                                         start=True, stop=True)
                        yc = ip.tile([1, CCOLS], f32, tag="yc", name=f"yc_{c}")
                        nc.vector.tensor_scalar(
                            out=yc[:1, :], in0=hp[:], scalar1=pbv[:1, :1],
                            scalar2=None, op0=mybir.AluOpType.add)
                        nc.sync.dma_start(
                            out=y_out[c * CCOLS:(c + 1) * CCOLS, :],
                            in_=yc[:1, :])
                if agin is not None:
                    nc.sync.dma_start(
                        out=agin.ap().rearrange("(k p) d -> p k d", p=128),
                        in_=rows[:])

            def allgather(nm):
                agin, agout = ag[nm]
                nc.gpsimd.collective_compute(
                    "AllGather",
                    mybir.AluOpType.bypass,
                    ins=[agin.ap().opt()],
                    outs=[agout.ap().opt()],
                    replica_groups=[list(range(NCORES))],
                )

            if mode == "spmm_only":
                for r in range(repeat):
                    spmm(feat_tbl, p1T, f"r{r}s1", nsqm2, ag["t2"][0])
                nc.sync.dma_start(out=y_out[:, :], in_=p1T[:1, :NCOLS])
                repeat = 0
            elif mode == "gather_only":
                for r in range(repeat):
                    for c in range(NCH):
                        ps = spmm_chunk(feat_tbl, c, f"r{r}")
                        nc.vector.tensor_copy(
                            out=p1T[:, c * CCOLS:(c + 1) * CCOLS], in_=ps[:])
                nc.sync.dma_start(out=y_out[:, :], in_=p1T[:1, :NCOLS])
                repeat = 0
            do_ag = mode != "noag"
            for r in range(repeat):
                # ---- layer 1 ----
                spmm(feat_tbl, p1T, f"r{r}s1", nsqm2, ag["t2"][0])
                if do_ag:
                    allgather("t2")
                t2 = ag["t2"][1] if do_ag else feat_tbl
                spmm_fused_dense(t2, 0, None, p1T, h1T, f"r{r}s2",
                                 agin=ag["t3"][0], scale=nsqp1)
                if do_ag:
                    allgather("t3")
                t3 = ag["t3"][1] if do_ag else feat_tbl
                # ---- layer 2 ----
                spmm(t3, p1T, f"r{r}s3", nsqm2, ag["t4"][0])
                if do_ag:
                    allgather("t4")
                t4 = ag["t4"][1] if do_ag else feat_tbl
                spmm_fused_dense(t4, 1, h1T, p1T, None, f"r{r}s4",
                                 head=True)

    _finalize_with_split(nc)
    return nc


def _meta_key(meta):
    return (meta["kc"].tobytes(), meta["NTILE_TOT"])


def _get_runner(meta):
    key = _meta_key(meta)
    if _CACHE.get("key") == key:
        return _CACHE["runner"]
    nc = _build_nc(meta)
    _CACHE["runner"] = _build_runner(nc, NCORES)
    _CACHE["key"] = key
    return _CACHE["runner"]


def kernel(features, src, dst, W, b, pw, pb):
    in_maps, meta, perms = _host_prep(features, src, dst, W, b, pw, pb)
    run = _get_runner(meta)
    results, times = run(in_maps, iters=1)
    _CACHE["last_times"] = times
    y = np.empty((N, 1), dtype=np.float32)
    for i in range(NCORES):
        y[i * NSH:(i + 1) * NSH, 0] = results[i]["y"][perms[i], 0]
    return y
